# revision 9
# baseline (speedup 1.0000x reference)
"""MoE routed matmul kernel for Trainium2 (8 NeuronCores, expert-parallel).

Problem: out[b, u] = sum_d x[b, d] * embeddings[content_idx[b], d, u]
with B=256 examples, D=U=1024, C=64 experts (256 MB fp32 table).

Strategy (expert parallel):
  - Core k owns experts [8k, 8k+8). It streams its 8 expert matrices
    (32 MB) from HBM once — the memory roofline for this problem.
  - The host groups examples by expert (pure index bookkeeping), packs
    each group into CAP padded slots, and lays the grouped x out in the
    exact transposed SBUF layout the PE wants (lhsT = x^T per k-chunk).
  - On device, per expert: out[slots, u] = sum_k xT_chunk.T @ W_chunk,
    accumulated in PSUM over 8 k-chunks of 128, with U split in two
    512-wide PSUM banks.
  - Host scatters the padded per-slot outputs back to example order.

The contraction index d is permuted as d = p*8 + b (p = partition,
b = k-chunk) identically on both x and W, which makes every weight DMA
read fully contiguous HBM (the host pre-lays the SBUF image).

Numerics ("bf16" variant, default): both x and W stream as plain
bf16 with fp32 PSUM accumulation. L2 relative error is ~2.4e-3 —
8x under the 2e-2 gate — while weight DMA traffic halves versus the
hi/lo-split "bf16x2" variant (16.8 MB vs 33.6 MB per core), which is
the dominant term since this kernel is HBM-bandwidth-bound at the
static ~435 GB/s per-core SBUF-fabric ceiling. "bf16x2" (hi+lo split,
~1e-6 rms) and "fp32" (exact, PE at 4 cycles/row) remain as fallback
variants.
"""

import numpy as np
import ml_dtypes

from concourse import bacc, mybir, tile
from concourse import bass_utils

BF16 = ml_dtypes.bfloat16
import os
FILL = os.environ.get("KFILL", "1") == "1"

B, D, U, C = 256, 1024, 1024, 64
NCORES = 8
EPC = C // NCORES          # experts per core
KC = D // 128              # k-chunks per expert
NCH = U // 512             # psum n-chunks per expert
NJ = U // 128              # u-chunks (weight-stationary 128-col tiles)
E3 = ml_dtypes.float8_e3m4

_compiled = {}


def _build_fp8ws(cap: int, xmode: str):
    """fp8-e3m4 weight-STATIONARY per-core SPMD program.

    The key inversion vs the bf16 variant: the weight chunk [128 d x
    128 u] is the PE's stationary operand (full array utilization +
    the compiler's Fast-Weight-Load path streams fp8 weights 4/cycle),
    and the tiny packed-x slab [128 d x cap2 slots] is the moving
    operand (cap2 ~ 24 cycles/matmul instead of 1024). PE time per
    expert drops from ~3.4 us to ~1 us, and weight DMA halves to
    1 MB/expert (e3m4), so the kernel tracks the ~358 GB/s per-core
    HBM roofline: ~8.4 MB -> ~24 us floor.

    xmode 'bf16': rhs is bf16 x (mixed-dtype matmul), single psum
    accumulation per (e, j), rel err ~1.3e-2.
    xmode 'f8hl': rhs is e3m4 [xh | xl*16] slot pairs; psum holds
    [hi | lo] column pairs per j, folded as hi + lo/16 on DVE.
    """
    f32 = mybir.dt.float32
    bf16 = mybir.dt.bfloat16
    f8 = mybir.dt.float8e3
    hl = xmode == "f8hl"
    cap2 = 2 * cap if hl else cap
    xdt = f8 if hl else bf16
    assert NJ * cap2 <= 512  # one psum bank per expert
    nc = bacc.Bacc("TRN2", target_bir_lowering=False, debug=False)
    wq = nc.dram_tensor("wq", [EPC, 128, KC * U], f8,
                        kind="ExternalInput").ap()
    xt = nc.dram_tensor("xt", [128, EPC * KC * cap2], xdt,
                        kind="ExternalInput").ap()
    out = nc.dram_tensor("out", [EPC, 128, NJ * cap], bf16,
                         kind="ExternalOutput").ap()

    HW = (KC // 2) * U     # half-expert weight columns (4096 = 512 KB)
    with tile.TileContext(nc) as tc:
        # wp bufs=16: ALL weight transfers are issued upfront so the HWDGE
        # rings never starve waiting on PE progress (64 KB/partition SBUF).
        with tc.tile_pool(name="wp", bufs=2 * EPC) as wp, \
             tc.tile_pool(name="xp", bufs=1) as xp, \
             tc.tile_pool(name="pp", bufs=6, space="PSUM") as pp, \
             tc.tile_pool(name="op", bufs=4) as op:
            xt_t = xp.tile([128, EPC * KC * cap2], xdt)
            # xt rides SWDGE (~2us fixed + 0.6us) so the HWDGE rings carry
            # nothing but the weight stream from t=0; it lands before the
            # first expert's weights do.
            nc.gpsimd.dma_start(xt_t[:], xt[:])

            held = []
            for e in range(EPC):
                wa = wp.tile([128, HW], f8, tag="wa")
                nc.sync.dma_start(wa[:], wq[e][:, :HW])
                wb_ = wp.tile([128, HW], f8, tag="wb")
                nc.scalar.dma_start(wb_[:], wq[e][:, HW:])
                ps = pp.tile([128, 512], f32)
                ot = op.tile([128, NJ * cap], bf16, tag="ot")
                tmp = op.tile([128, NJ * cap], f32, tag="tmp")
                last = e == EPC - 1
                # j OUTER, b inner: the PSUM has_written mask that `start`
                # resets is bank-wide, so only one accumulation group may
                # be open per bank at a time. Completed groups' VALUES
                # survive later groups' starts; only the mask resets.
                for j in range(NJ):
                    for b in range(KC):
                        wc, bl = (wa, b) if b < KC // 2 else (wb_, b - KC // 2)
                        fo = (e * KC + b) * cap2
                        nc.tensor.matmul(
                            ps[:, j * cap2: j * cap2 + cap2],
                            lhsT=wc[:, bl * U + j * 128: bl * U + j * 128 + 128],
                            rhs=xt_t[:, fo: fo + cap2],
                            start=(b == 0),
                            stop=(b == KC - 1),
                        )
                    if last:
                        # last expert: fold each group the moment it stops
                        # (overlaps the remaining matmuls and keeps DVE
                        # awake), and stage the store in two ring-split
                        # halves so only a 12 KB store trails the last
                        # matmul.
                        cj = slice(j * cap, (j + 1) * cap)
                        if hl:
                            nc.vector.tensor_scalar_mul(
                                tmp[:, cj],
                                ps[:, j * cap2 + cap: j * cap2 + cap2],
                                1.0 / 16)
                            nc.vector.tensor_add(
                                ot[:, cj], tmp[:, cj],
                                ps[:, j * cap2: j * cap2 + cap])
                        else:
                            nc.vector.tensor_copy(
                                ot[:, cj], ps[:, j * cap2: j * cap2 + cap2])
                        if j == NJ // 2 - 1:
                            h1 = NJ // 2 * cap
                            nc.sync.dma_start(out[e][:, :h1], ot[:, :h1])
                        elif j == NJ - 1:
                            h1 = NJ // 2 * cap
                            nc.scalar.dma_start(out[e][:, h1:], ot[:, h1:])
                if not last:
                    if hl:
                        psv = ps[:, :NJ * cap2].rearrange(
                            "p (j t c) -> p j t c", t=2, c=cap)
                        tmpv = tmp[:].rearrange(
                            "p (j t c) -> p j t c", t=1, c=cap)
                        otv = ot[:].rearrange(
                            "p (j t c) -> p j t c", t=1, c=cap)
                        nc.vector.tensor_scalar_mul(
                            tmpv, psv[:, :, 1:2, :], 1.0 / 16)
                        nc.vector.tensor_add(otv, tmpv, psv[:, :, 0:1, :])
                    else:
                        nc.vector.tensor_copy(ot[:], ps[:, :NJ * cap])
                    nc.gpsimd.dma_start(out[e], ot[:])
    nc.compile()
    return nc


def _build_bf16(cap: int):
    """Single-bf16 per-core SPMD program (PE at 1 cycle/row, half the
    weight bytes of bf16x2).

    The 2e-2 harness gate leaves ~8x margin over the ~2.4e-3 L2 error
    of a bf16xbf16 matmul with fp32 PSUM accumulation, so both x and W
    stream as plain bf16: 2 MB per expert instead of 4 MB (~17.5 MB
    per core total). Both HWDGE rings sustain ~210 GB/s each — the
    ~435 GB/s SBUF-fabric ceiling combined — which makes the kernel
    DMA-bound end to end; everything below is about keeping both rings
    streaming continuously and keeping the PE out of HAM throttle.

    Layout: expert e < 7 streams k-chunks 0-3 on the sync ring and
    4-7 on the scalar ring (1 MB chunks), so each expert completes
    ~5 us after the previous and the PE (3.4 us/expert warm) keeps
    pace. Expert 7 is tapered into 4 x 512 KB k-pair granules, ring-
    alternated, so after the last granule lands only 4 matmuls + the
    PSUM fold + one 64 KB store remain. The fold is split across
    DVE (j=0 bank) and ACT (j=1 bank) — both can read PSUM. xt rides
    the gpsimd SWDGE queue to keep the HWDGE rings free for weights.
    Zero-filler matmuls (+0 accumulates of an all-zero rhs) pad each
    expert's PE burst so idle gaps never cross the ~3.4 us HAM window
    that would halve the PE clock right before the tail.
    """
    f32 = mybir.dt.float32
    bf16 = mybir.dt.bfloat16
    nc = bacc.Bacc("TRN2", target_bir_lowering=False, debug=False)
    wb = nc.dram_tensor("wb", [EPC, 128, KC * U], bf16,
                        kind="ExternalInput").ap()
    xt = nc.dram_tensor("xt", [128, EPC * KC * cap], bf16,
                        kind="ExternalInput").ap()
    out = nc.dram_tensor("out", [EPC, cap, U], bf16,
                         kind="ExternalOutput").ap()

    ET = EPC - 1   # the tapered last expert
    with tile.TileContext(nc) as tc:
        with tc.tile_pool(name="wpa", bufs=6) as wpa, \
             tc.tile_pool(name="wpb", bufs=KC // 2) as wpb, \
             tc.tile_pool(name="xp", bufs=1) as xp, \
             tc.tile_pool(name="pp", bufs=3, space="PSUM") as pp, \
             tc.tile_pool(name="op", bufs=3) as op:
            xt_t = xp.tile([128, EPC * KC * cap], bf16)
            nc.gpsimd.dma_start(xt_t[:], xt[:])
            # all-zero rhs for PE-warming filler matmuls (+0 accumulate)
            zt = xp.tile([128, 512], bf16, tag="z")
            nc.gpsimd.memzero(zt[:])

            # chunk issues are emitted before any fold so the engine
            # queues never block an issue behind PSUM work; wpa's
            # recycling paces issues so the framework's small rotating
            # pool (~9) of DMA-completion semaphores never serializes an
            # issue against an unfinished transfer.
            half = KC // 2
            chunks = {}
            for e in range(ET):
                ca = wpa.tile([128, half * U], bf16, tag="ca")
                nc.sync.dma_start(ca[:], wb[e][:, :half * U])
                cb = wpa.tile([128, half * U], bf16, tag="cb")
                nc.scalar.dma_start(cb[:], wb[e][:, half * U:])
                chunks[e] = lambda b, ca=ca, cb=cb: (
                    (ca, b) if b < half else (cb, b - half))

            held = []
            for e in range(EPC):
                if e == EPC - 3:
                    # the tapered last expert's 512 KB k-pair granules are
                    # issued here: late enough that their rotated
                    # semaphores' previous users have completed, early
                    # enough to be queued well before the rings drain to
                    # them. After the last granule lands only 4 matmuls +
                    # the fold + one 64 KB store remain.
                    gr = []
                    for p in range(KC // 2):
                        g = wpb.tile([128, 2 * U], bf16, tag="g")
                        eng = nc.sync if p % 2 == 0 else nc.scalar
                        eng.dma_start(
                            g[:], wb[ET][:, 2 * p * U:2 * (p + 1) * U])
                        gr.append(g)
                    chunks[ET] = gr
                for m0 in range(0, cap, 128):
                    mm = min(128, cap - m0)
                    ps = pp.tile([mm, U], f32)
                    fo0 = e * KC * cap + m0

                    def filler(n):
                        # +0 accumulates that only keep the PE busy while
                        # the next weight chunk is in flight, so idle gaps
                        # never cross HAM's ~3.4 us re-throttle window
                        for _ in range(n):
                            nc.tensor.matmul(
                                ps[:, :512], lhsT=xt_t[:, fo0:fo0 + mm],
                                rhs=zt[:], start=False, stop=False)

                    for b in range(KC):
                        c = chunks[e]
                        wc, bl = c(b) if callable(c) else (c[b // 2], b % 2)
                        fo = e * KC * cap + b * cap + m0
                        if FILL and e < ET and b == KC - 1:
                            filler(4)
                        for j in range(NCH):
                            nc.tensor.matmul(
                                ps[:, j * 512:(j + 1) * 512],
                                lhsT=xt_t[:, fo:fo + mm],
                                rhs=wc[:, bl * U + j * 512:
                                       bl * U + j * 512 + 512],
                                start=(b == 0),
                                stop=(b == KC - 1),
                            )
                    ot = op.tile([mm, U], bf16, tag="ot")
                    if e == ET:
                        # single-engine fold: keeps the scalar engine's
                        # ~1us wake latency out of the final chain
                        nc.vector.tensor_copy(ot[:], ps[:])
                    else:
                        nc.vector.tensor_copy(ot[:, :512], ps[:, :512])
                        nc.scalar.copy(ot[:, 512:], ps[:, 512:])
                    if e < EPC - 2:
                        nc.gpsimd.dma_start(out[e, m0:m0 + mm, :], ot[:])
                    else:
                        held.append((e, m0, mm, ot))
            for i, (e, m0, mm, ot) in enumerate(held):
                eng = nc.sync if i % 2 == 0 else nc.scalar
                eng.dma_start(out[e, m0:m0 + mm, :], ot[:])
    nc.compile()
    return nc


def _build_fp32(cap: int):
    """Exact-fp32 per-core SPMD program (PE at 4 cycles/row)."""
    f32 = mybir.dt.float32
    nc = bacc.Bacc("TRN2", target_bir_lowering=False, debug=False)
    w = nc.dram_tensor("w", [EPC, D, U], f32, kind="ExternalInput").ap()
    xt = nc.dram_tensor("xt", [128, EPC * KC * cap], f32, kind="ExternalInput").ap()
    out = nc.dram_tensor("out", [EPC, cap, U], f32, kind="ExternalOutput").ap()

    with tile.TileContext(nc) as tc:
        with tc.tile_pool(name="wp", bufs=2) as wp, \
             tc.tile_pool(name="xp", bufs=1) as xp, \
             tc.tile_pool(name="pp", bufs=4, space="PSUM") as pp, \
             tc.tile_pool(name="op", bufs=3) as op:
            xt_t = xp.tile([128, EPC * KC * cap], f32)
            nc.sync.dma_start(xt_t[:], xt[:])
            for e in range(EPC):
                # whole expert weight as [128, KC*U]; d = p*KC + b, so the
                # HBM read is fully contiguous per partition (32 KB).
                w_t = wp.tile([128, KC * U], f32)
                nc.sync.dma_start(
                    w_t[:].rearrange("p (b u) -> p b u", b=KC),
                    w[e].rearrange("(p b) u -> p b u", b=KC),
                )
                for m0 in range(0, cap, 128):
                    mm = min(128, cap - m0)
                    ps = pp.tile([mm, U], f32)
                    for j in range(NCH):
                        for b in range(KC):
                            fo = e * KC * cap + b * cap + m0
                            nc.tensor.matmul(
                                ps[:, j * 512:(j + 1) * 512],
                                lhsT=xt_t[:, fo:fo + mm],
                                rhs=w_t[:, b * U + j * 512: b * U + j * 512 + 512],
                                start=(b == 0),
                                stop=(b == KC - 1),
                            )
                    ot = op.tile([mm, U], f32)
                    nc.vector.tensor_copy(ot[:], ps[:])
                    nc.sync.dma_start(out[e, m0:m0 + mm, :], ot[:])
    nc.compile()
    return nc


def _build_bf16x2(cap: int):
    """bf16 hi/lo split per-core SPMD program (PE at 1 cycle/row).

    whl holds the host-prepared SBUF image: whl[e, p, (2b+wi)*U + u] =
    W_wi[d = p*KC + b, u] (wi: 0=hi, 1=lo). lhsT layout per (e, b):
    2*cap columns = [xh slots | xl slots]. Each psum n-chunk is one
    accumulation group of 2*KC matmuls; row i collects xh_i@(Wh+Wl),
    row cap+i collects xl_i@(Wh+Wl), and a DVE copy+add folds them.
    """
    f32 = mybir.dt.float32
    bf16 = mybir.dt.bfloat16
    cap2 = 2 * cap
    assert cap2 <= 128 and cap % 32 == 0
    NBP = 4        # DMA chunks per expert (1 MB each)
    BPK = KC // NBP  # k-chunks per DMA chunk
    nc = bacc.Bacc("TRN2", target_bir_lowering=False, debug=False)
    whl = nc.dram_tensor("whl", [EPC, 128, KC * 2 * U], bf16,
                         kind="ExternalInput").ap()
    xt = nc.dram_tensor("xt", [128, EPC * KC * cap2], bf16,
                        kind="ExternalInput").ap()
    out = nc.dram_tensor("out", [EPC, cap, U], f32, kind="ExternalOutput").ap()

    with tile.TileContext(nc) as tc:
        with tc.tile_pool(name="wp", bufs=3 * NBP + 2) as wp, \
             tc.tile_pool(name="xp", bufs=1) as xp, \
             tc.tile_pool(name="pp", bufs=4, space="PSUM") as pp, \
             tc.tile_pool(name="op", bufs=3) as op:
            xt_t = xp.tile([128, EPC * KC * cap2], bf16)
            # xt must land before the first matmul: SWDGE would take ~15us
            # (1KB packets), so split it across both HWDGE rings ahead of
            # the weight stream (~1.5us each)
            half = EPC * KC * cap2 // 2
            nc.sync.dma_start(xt_t[:, :half], xt[:, :half])
            nc.scalar.dma_start(xt_t[:, half:], xt[:, half:])
            held = []
            for e in range(EPC):
                chunks = []
                for bp in range(NBP):
                    wc = wp.tile([128, 2 * BPK * U], bf16, tag="wc")
                    # alternate the two HWDGE rings (SP + ACT) so weight
                    # streams use both hardware queues
                    eng = nc.sync if (e * NBP + bp) % 2 == 0 else nc.scalar
                    eng.dma_start(
                        wc[:],
                        whl[e][:, bp * 2 * BPK * U:(bp + 1) * 2 * BPK * U],
                    )
                    chunks.append(wc)
                ps = pp.tile([cap2, U], f32)
                for bp in range(NBP):
                    wc = chunks[bp]
                    for bl in range(BPK):
                        b = bp * BPK + bl
                        fo = e * KC * cap2 + b * cap2
                        for wi in range(2):
                            for j in range(NCH):
                                nc.tensor.matmul(
                                    ps[:, j * 512:(j + 1) * 512],
                                    lhsT=xt_t[:, fo:fo + cap2],
                                    rhs=wc[:, (2 * bl + wi) * U + j * 512:
                                            (2 * bl + wi) * U + j * 512 + 512],
                                    start=(bp == 0 and bl == 0 and wi == 0),
                                    stop=(bp == NBP - 1 and bl == BPK - 1
                                          and wi == 1),
                                )
                # fold the two slot halves. DVE may read only one PSUM
                # operand per op: copy hi out, then add lo.
                tmp = op.tile([cap, U], f32, tag="tmp")
                ot = op.tile([cap, U], f32, tag="ot")
                nc.vector.tensor_copy(tmp[:], ps[:cap, :])
                nc.vector.tensor_add(ot[:], tmp[:], ps[cap:cap2, :])
                if e < EPC - 2:
                    # mid-stream outputs ride SWDGE so the HWDGE rings
                    # stay clear for the weight stream
                    nc.gpsimd.dma_start(out[e, :, :], ot[:])
                else:
                    # last two experts' outputs go at the end on the
                    # by-then-idle HWDGE rings (SWDGE is ~2us/DMA and
                    # would stretch the tail)
                    held.append((e, ot))
            for (e, ot), eng in zip(held, (nc.sync, nc.scalar)):
                eng.dma_start(out[e, :, :], ot[:])
    nc.compile()
    return nc


def _get_compiled(cap: int, variant: str):
    key = (cap, variant)
    if key not in _compiled:
        if variant == "fp32":
            _compiled[key] = _build_fp32(cap)
        elif variant == "bf16x2":
            _compiled[key] = _build_bf16x2(cap)
        elif variant == "bf16":
            _compiled[key] = _build_bf16(cap)
        elif variant.startswith("fp8ws"):
            _compiled[key] = _build_fp8ws(cap, variant.split("_")[1])
        else:
            raise ValueError(variant)
    return _compiled[key]


def _pow2_scale(absmax: float) -> float:
    """Largest power of 2 s s.t. absmax * s <= 15.5 (e3m4 max normal)."""
    s = 1.0
    while absmax * s * 2 <= 15.5:
        s *= 2
    while absmax * s > 15.5:
        s /= 2
    return s


def _route(content_idx, x, cap):
    """Group examples by expert into padded slots. Returns the packed
    per-expert x [C, cap, D] plus the (expert, slot) of every example."""
    counts = np.bincount(content_idx, minlength=C)
    order = np.argsort(content_idx, kind="stable")
    cs = content_idx[order]
    starts = np.zeros(C, np.int64)
    starts[1:] = np.cumsum(counts)[:-1]
    slot = np.arange(B) - starts[cs]
    xp_ = np.zeros((C, cap, D), np.float32)
    xp_[cs, slot] = x[order]
    return xp_, order, cs, slot


def _to_lhsT(xp_, cap, dtype):
    """[C, cap, D] packed x -> per-core lhsT layout
    [NCORES, 128, EPC*KC*cap] with free index e*KC*cap + b*cap + i and
    the d = p*KC + b permutation (matching the weight layout)."""
    xt = np.asarray(xp_, dtype).reshape(C, cap, 128, KC)  # [c, i, p, b]
    xt = xt.reshape(NCORES, EPC, cap, 128, KC)
    xt = xt.transpose(0, 3, 1, 4, 2)                      # [k, p, e, b, i]
    return np.ascontiguousarray(xt).reshape(NCORES, 128, EPC * KC * cap)


def _run_fp8ws(content_idx, x, embeddings, xmode, trace, trace_cores):
    counts = np.bincount(content_idx, minlength=C)
    cap = max(4, -(-int(counts.max()) // 4) * 4)
    hl = xmode == "f8hl"
    if NJ * cap * (2 if hl else 1) > 512:
        return None  # pathological skew; caller falls back
    xp_, order, cs, slot = _route(content_idx, x, cap)

    sw = _pow2_scale(float(np.abs(embeddings).max()))
    # wq[e, p, b*U + u] = e3m4(W[d = p*KC + b, u] * sw): same contiguous
    # d-permuted HBM layout as the bf16 variant, 1 byte/elem.
    wq = np.ascontiguousarray(
        (embeddings * sw).astype(E3).reshape(C, 128, KC * U))
    if hl:
        sx = _pow2_scale(float(np.abs(xp_).max()))
        xh = (xp_ * sx).astype(E3)
        xl = ((xp_ * sx - xh.astype(np.float32)) * 16).astype(E3)
        xq = np.concatenate([xh, xl], axis=1)       # [C, 2cap, D]
        xt = _to_lhsT(xq, 2 * cap, E3)
    else:
        sx = 1.0
        xt = _to_lhsT(xp_, cap, BF16)

    nc = _get_compiled(cap, "fp8ws_" + xmode)
    in_maps = [
        {"wq": wq[k * EPC:(k + 1) * EPC], "xt": xt[k]}
        for k in range(NCORES)
    ]
    res = bass_utils.run_bass_kernel_spmd(
        nc, in_maps, core_ids=list(range(NCORES)),
        trace=trace, trace_cores=trace_cores,
    )
    outs = np.stack([res.results[k]["out"] for k in range(NCORES)])
    # outs [NCORES, EPC, 128, NJ*cap]: u = j*128 + p
    outs = outs.astype(np.float32).reshape(C, 128, NJ, cap)
    outs = outs.transpose(0, 2, 1, 3).reshape(C, U, cap)
    out = np.empty((B, U), np.float32)
    out[order] = outs[cs, :, slot] * (1.0 / (sx * sw))
    return out, res


def run(content_idx, x, embeddings, trace=False, trace_cores=None,
        variant="bf16"):
    content_idx = np.asarray(content_idx, np.int32)
    x = np.ascontiguousarray(np.asarray(x, np.float32))
    embeddings = np.ascontiguousarray(np.asarray(embeddings, np.float32))

    if variant.startswith("fp8ws"):
        r = _run_fp8ws(content_idx, x, embeddings, variant.split("_")[1],
                       trace, trace_cores)
        if r is not None:
            return r
        variant = "bf16"  # fallback for pathological expert skew

    counts = np.bincount(content_idx, minlength=C)
    cap = max(16, -(-int(counts.max()) // 16) * 16)
    if variant == "bf16":
        # slot padding only needs 4-alignment here (no partition-offset
        # reads); a tighter cap shrinks the xt and out streams
        cap = max(8, -(-int(counts.max()) // 4) * 4)
    if variant == "bf16x2":
        # DVE partition access is 32-granular (the lo half starts at
        # partition cap) and stacked [xh; xl] needs 2*cap <= 128.
        cap = max(32, -(-int(counts.max()) // 32) * 32)
        if cap > 64:
            variant = "fp32"
            cap = max(16, -(-int(counts.max()) // 16) * 16)
    xp_, order, cs, slot = _route(content_idx, x, cap)

    nc = _get_compiled(cap, variant)
    if variant == "fp32":
        xt = _to_lhsT(xp_, cap, np.float32)
        in_maps = [
            {"w": embeddings[k * EPC:(k + 1) * EPC], "xt": xt[k]}
            for k in range(NCORES)
        ]
    elif variant == "bf16":
        # wb[e, p, b*U + u] = W_bf16[d = p*KC + b, u]: contiguous HBM
        # reads per partition, same d-permutation as the lhsT.
        wb = np.ascontiguousarray(
            embeddings.astype(BF16).reshape(C, 128, KC * U))
        xt = _to_lhsT(xp_, cap, BF16)
        in_maps = [
            {"wb": wb[k * EPC:(k + 1) * EPC], "xt": xt[k]}
            for k in range(NCORES)
        ]
    else:
        w_hi = embeddings.astype(BF16)
        w_lo = (embeddings - w_hi.astype(np.float32)).astype(BF16)
        # SBUF image: [c, p, b, wi, u] contiguous; d = p*KC + b
        whl = np.stack(
            [w_hi.reshape(C, 128, KC, U), w_lo.reshape(C, 128, KC, U)],
            axis=3,
        ).reshape(C, 128, KC * 2 * U)
        x_hi = xp_.astype(BF16)
        x_lo = (xp_ - x_hi.astype(np.float32)).astype(BF16)
        xhl = np.concatenate([x_hi, x_lo], axis=1)  # [C, 2*cap, D]
        xt = _to_lhsT(xhl, 2 * cap, BF16)
        in_maps = [
            {"whl": whl[k * EPC:(k + 1) * EPC], "xt": xt[k]}
            for k in range(NCORES)
        ]

    res = bass_utils.run_bass_kernel_spmd(
        nc, in_maps, core_ids=list(range(NCORES)),
        trace=trace, trace_cores=trace_cores,
    )
    outs = np.stack([res.results[k]["out"] for k in range(NCORES)])
    outs = outs.astype(np.float32).reshape(C, cap, U)
    out = np.empty((B, U), np.float32)
    out[order] = outs[cs, slot]
    return out, res


def kernel(content_idx, x, embeddings):
    out, _ = run(content_idx, x, embeddings)
    return out

